# revision 4
# baseline (speedup 1.0000x reference)
"""AtomAttentionPairBias — window-sharded across 8 NeuronCores.

Sharding: 512 windows -> 64 windows per core (sequence-parallel over atoms
with a 48-atom halo per side, per the sharding hint). Shards are independent
given the halo: no collectives.

Wall-clock here is dominated by the axon tunnel, which caps at ~60 MB/s
*per connection* but scales with the number of connections. So kernel()
runs a persistent pool of 8 worker processes, each owning one NeuronCore
through its own jax client/connection (~8x aggregate wire bandwidth). The
main process casts inputs to bf16 into POSIX shared memory (measured
3.5e-3 output rel err vs the 2e-2 budget); workers slice their halo
shard zero-copy, upload, execute, and write their f32 output slice back
into shared memory.

If the pool cannot be set up, a single-connection in-process fallback
path (same math) is used.

Hardcoded shapes (self-contained; must not read spec/reference):
  atom_single/atom_proj: [1, 4, 16384, 128] f32
  atom_pair_local:       [1, 512, 32, 128, 16] f32
  mask:                  [1, 16384] f32
"""

import os
import sys
import time
import traceback
import numpy as np
import ml_dtypes

C_ATOM = 128
C_PAIR = 16
H = 4
CH = C_ATOM // H
NQ = 32
NK = 128
INF = 1e8
BS, S, N = 1, 4, 16384
P = N // NQ           # 512 windows
NCORES = 8
WC = P // NCORES      # 64 windows per core
AC = N // NCORES      # 2048 atoms per core
PAD = (NK - NQ) // 2  # 48 halo atoms
AH = 2176             # halo shard padded up to 17*128 (>= AC + 2*PAD = 2144)
NPADDED = AC * (NCORES - 1) + AH + PAD  # xs buffer length so every slice fits

BF16 = ml_dtypes.bfloat16

_TIMING = bool(os.environ.get("KERNEL_DEBUG_TIMING"))

SHM_PREFIX = f"aapb_{os.getpid()}_"

XS_SHAPE = (S, NPADDED, C_ATOM)          # bf16
PAIR_SHAPE = (P, NQ, NK, C_PAIR)         # bf16
MB_SHAPE = (P, NK)                       # f32
OUT_SHAPE = (BS, S, N, C_ATOM)           # f32
W_BYTES = 4 * (C_ATOM + 3 * C_ATOM * C_ATOM + 4 * C_ATOM * CH * H
               + 2 * C_PAIR + C_PAIR * H + 4 * C_ATOM)  # generous upper bound

W_SPECS = [
    ("adaln_s_scale", (C_ATOM,)), ("w_gate", (C_ATOM, C_ATOM)),
    ("b_gate", (C_ATOM,)), ("w_skip", (C_ATOM, C_ATOM)),
    ("wq", (C_ATOM, C_ATOM)), ("wk", (C_ATOM, C_ATOM)),
    ("wv", (C_ATOM, C_ATOM)), ("wg", (C_ATOM, C_ATOM)),
    ("bg", (C_ATOM,)), ("wo", (C_ATOM, C_ATOM)), ("bo", (C_ATOM,)),
    ("pair_ln_scale", (C_PAIR,)), ("pair_ln_bias", (C_PAIR,)),
    ("w_pair", (C_PAIR, H)), ("w_out", (C_ATOM, C_ATOM)), ("b_out", (C_ATOM,)),
]


def _tlog(msg):
    if _TIMING:
        print(f"[kernel {time.perf_counter():.3f}] {msg}", flush=True)


# ---------------------------------------------------------------- shard math
def build_shard_fn():
    """Per-core computation: [S, AH, C] bf16 inputs -> [S, AC, C] bf16."""
    import jax
    import jax.numpy as jnp

    f32 = jnp.float32

    def _ln(x, eps=1e-5):
        mu = jnp.mean(x, axis=-1, keepdims=True)
        var = jnp.var(x, axis=-1, keepdims=True)
        return (x - mu) * jax.lax.rsqrt(var + eps)

    def shard_fn(xs, xp, pair, mb,
                 adaln_s_scale, w_gate, b_gate, w_skip,
                 wq, wk, wv, wg, bg, wo, bo,
                 pair_ln_scale, pair_ln_bias, w_pair, w_out, b_out):
        bf16 = jnp.bfloat16
        xs = xs.astype(f32)
        xp = xp.astype(f32)
        a = _ln(xs)
        sp = _ln(xp) * adaln_s_scale
        spb = sp.astype(bf16)
        gate = jax.nn.sigmoid((spb @ w_gate.astype(bf16)).astype(f32) + b_gate)
        a = gate * a + (spb @ w_skip.astype(bf16)).astype(f32)

        idx_k = jnp.arange(WC)[:, None] * NQ + jnp.arange(NK)[None, :]
        idx_q = PAD + jnp.arange(WC)[:, None] * NQ + jnp.arange(NQ)[None, :]
        ab = a.astype(bf16)
        kvx = ab[:, idx_k]       # [S, WC, NK, C] bf16
        qx = ab[:, idx_q]        # [S, WC, NQ, C] bf16

        lb = (_ln(pair.astype(f32)) * pair_ln_scale + pair_ln_bias)
        lb = lb.astype(bf16) @ w_pair.astype(bf16)        # [WC,NQ,NK,H]
        pb = jnp.transpose(lb.astype(f32), (0, 3, 1, 2))  # [WC,H,NQ,NK]

        q = (qx @ wq.astype(bf16)).reshape(S, WC, NQ, H, CH)
        k = (kvx @ wk.astype(bf16)).reshape(S, WC, NK, H, CH)
        v = (kvx @ wv.astype(bf16)).reshape(S, WC, NK, H, CH)
        scores = jnp.einsum('swqhc,swkhc->swhqk', q, k,
                            preferred_element_type=f32) / jnp.sqrt(f32(CH))
        scores = scores + mb[None, :, None, None, :] + pb[None]
        att = jax.nn.softmax(scores, axis=-1)
        o = jnp.einsum('swhqk,swkhc->swqhc', att.astype(bf16), v,
                       preferred_element_type=f32).reshape(S, WC, NQ, H * CH)
        og = jax.nn.sigmoid((qx @ wg.astype(bf16)).astype(f32) + bg) * o
        o2 = (og.astype(bf16) @ wo.astype(bf16)).astype(f32) + bo
        out = jax.nn.sigmoid((o2.astype(bf16) @ w_out.astype(bf16)).astype(f32)
                             + b_out) * o2
        return out.reshape(S, AC, C_ATOM).astype(bf16)

    return shard_fn


def unpack_weights(buf):
    """f32 byte buffer -> list of weight arrays per W_SPECS."""
    out = []
    off = 0
    flat = np.frombuffer(buf, np.float32)
    for _, shape in W_SPECS:
        n = int(np.prod(shape))
        out.append(flat[off:off + n].reshape(shape).copy())
        off += n
    return out


# ------------------------------------------------------------------- worker
def worker_main(idx, conn, shm_names):
    """Persistent per-core worker: owns jax client + device idx."""
    try:
        from multiprocessing import shared_memory
        shms = {k: shared_memory.SharedMemory(name=v) for k, v in shm_names.items()}
        xs_v = np.ndarray(XS_SHAPE, BF16, buffer=shms['xs'].buf)
        xp_v = np.ndarray(XS_SHAPE, BF16, buffer=shms['xp'].buf)
        pair_v = np.ndarray(PAIR_SHAPE, BF16, buffer=shms['pair'].buf)
        mb_v = np.ndarray(MB_SHAPE, np.float32, buffer=shms['mb'].buf)
        out_v = np.ndarray(OUT_SHAPE, np.float32, buffer=shms['out'].buf)

        import jax
        dev = jax.devices()[idx]
        fn = jax.jit(build_shard_fn())
        conn.send(("ready", idx))

        lo = idx * AC  # shard start in padded coords
        wkey = None
        wdev = None
        state = {}

        while True:
            msg = conn.recv()
            kind = msg[0]
            if kind == "stop":
                break
            elif kind == "xs":
                state['xs'] = jax.device_put(
                    np.ascontiguousarray(xs_v[:, lo:lo + AH]), dev)
            elif kind == "xp":
                state['xp'] = jax.device_put(
                    np.ascontiguousarray(xp_v[:, lo:lo + AH]), dev)
            elif kind == "pair":
                state['pair'] = jax.device_put(
                    np.ascontiguousarray(pair_v[idx * WC:(idx + 1) * WC]), dev)
            elif kind == "go":
                new_wkey = msg[1]
                if new_wkey != wkey:
                    w_np = unpack_weights(shms['w'].buf[:new_wkey[0]])
                    wdev = [jax.device_put(w, dev) for w in w_np]
                    wkey = new_wkey
                mb_d = jax.device_put(
                    np.ascontiguousarray(mb_v[idx * WC:(idx + 1) * WC]), dev)
                out_dev = fn(state['xs'], state['xp'], state['pair'], mb_d, *wdev)
                out_np = np.asarray(out_dev)           # [S, AC, C] bf16
                out_v[0, :, idx * AC:(idx + 1) * AC, :] = out_np.astype(np.float32)
                conn.send(("done", idx))
    except Exception:
        try:
            conn.send(("error", idx, traceback.format_exc()))
        except Exception:
            pass


# ------------------------------------------------------------------- pool
class _Pool:
    def __init__(self):
        import multiprocessing as mp
        from multiprocessing import shared_memory

        ctx = mp.get_context("spawn")
        self.shms = {}
        for key, nbytes in [
            ("xs", int(np.prod(XS_SHAPE)) * 2),
            ("xp", int(np.prod(XS_SHAPE)) * 2),
            ("pair", int(np.prod(PAIR_SHAPE)) * 2),
            ("mb", int(np.prod(MB_SHAPE)) * 4),
            ("w", W_BYTES),
            ("out", int(np.prod(OUT_SHAPE)) * 4),
        ]:
            self.shms[key] = shared_memory.SharedMemory(
                create=True, size=nbytes, name=f"{SHM_PREFIX}{key}")
        names = {k: v.name for k, v in self.shms.items()}

        self.xs_v = np.ndarray(XS_SHAPE, BF16, buffer=self.shms['xs'].buf)
        self.xp_v = np.ndarray(XS_SHAPE, BF16, buffer=self.shms['xp'].buf)
        self.pair_v = np.ndarray(PAIR_SHAPE, BF16, buffer=self.shms['pair'].buf)
        self.mb_v = np.ndarray(MB_SHAPE, np.float32, buffer=self.shms['mb'].buf)
        self.out_v = np.ndarray(OUT_SHAPE, np.float32, buffer=self.shms['out'].buf)

        self.conns = []
        self.procs = []
        for i in range(NCORES):
            parent, child = ctx.Pipe()
            p = ctx.Process(target=worker_main, args=(i, child, names),
                            daemon=True, name=f"aapb-worker-{i}")
            p.start()
            self.conns.append(parent)
            self.procs.append(p)
        deadline = time.time() + 600
        for c in self.conns:
            if not c.poll(max(1.0, deadline - time.time())):
                raise RuntimeError("worker init timeout")
            msg = c.recv()
            if msg[0] != "ready":
                raise RuntimeError(f"worker failed: {msg}")
        _tlog("pool ready")

    def broadcast(self, msg):
        for c in self.conns:
            c.send(msg)

    def wait_done(self, timeout=900):
        deadline = time.time() + timeout
        for c in self.conns:
            if not c.poll(max(1.0, deadline - time.time())):
                raise RuntimeError("worker exec timeout")
            msg = c.recv()
            if msg[0] != "done":
                raise RuntimeError(f"worker error: {msg}")

    def alive(self):
        return all(p.is_alive() for p in self.procs)


_G: dict = {}


def _cast_bf16_into(dst, src):
    """f32 -> bf16 elementwise into a preallocated bf16 view."""
    dst[...] = src.astype(BF16)


def _run_pool(pool, atom_single, atom_proj, atom_pair_local, mask, weights):
    # Stage + signal per tensor so worker uploads overlap later host casts.
    xs = np.asarray(atom_single, np.float32)[0]
    pool.xs_v[:, PAD:PAD + N] = xs.astype(BF16)
    pool.xs_v[:, :PAD] = 0
    pool.xs_v[:, PAD + N:] = 0
    pool.broadcast(("xs",))
    _tlog("xs staged")

    xp = np.asarray(atom_proj, np.float32)[0]
    pool.xp_v[:, PAD:PAD + N] = xp.astype(BF16)
    pool.xp_v[:, :PAD] = 0
    pool.xp_v[:, PAD + N:] = 0
    pool.broadcast(("xp",))
    _tlog("xp staged")

    pr = np.asarray(atom_pair_local, np.float32)[0]
    # chunked cast so the first workers' uploads start sooner
    qtr = P // 4
    for c in range(4):
        pool.pair_v[c * qtr:(c + 1) * qtr] = pr[c * qtr:(c + 1) * qtr].astype(BF16)
    pool.broadcast(("pair",))
    _tlog("pair staged")

    mp_ = np.zeros((NPADDED,), np.float32)
    mp_[PAD:PAD + N] = np.asarray(mask, np.float32)[0]
    idx = np.arange(P)[:, None] * NQ + np.arange(NK)[None, :]
    pool.mb_v[...] = INF * (mp_[idx] - 1.0)

    wflat = np.concatenate([w.reshape(-1) for w in weights]).astype(np.float32)
    nb = wflat.nbytes
    pool.shms['w'].buf[:nb] = wflat.tobytes()
    wkey = (nb, float(wflat[0]), float(wflat[-1]),
            float(wflat.sum(dtype=np.float64)))
    pool.broadcast(("go", wkey))
    _tlog("go sent")

    pool.wait_done()
    _tlog("all done")
    return pool.out_v.copy()


# ---------------------------------------------------- in-process fallback
def _run_fallback(atom_single, atom_proj, atom_pair_local, mask, weights):
    import jax
    from jax.sharding import Mesh, NamedSharding, PartitionSpec as PS
    from jax.experimental.shard_map import shard_map

    if 'fb_fn' not in _G:
        devs = jax.devices()
        if len(devs) >= NCORES:
            mesh = Mesh(np.asarray(devs[:NCORES]), ("core",))
            base = build_shard_fn()

            def wrapped(xs, xp, pair, mb, *w):
                return base(xs[0], xp[0], pair, mb, *w)

            in_specs = (PS("core"),) * 4 + (PS(),) * 16
            f = shard_map(wrapped, mesh=mesh, in_specs=in_specs,
                          out_specs=PS(None, "core"), check_rep=False)
            _G['fb_fn'] = jax.jit(f)
            _G['fb_shard'] = NamedSharding(mesh, PS("core"))
            _G['fb_repl'] = NamedSharding(mesh, PS())
            _G['fb_multi'] = True
        else:
            _G['fb_fn'] = jax.jit(build_shard_fn())
            _G['fb_multi'] = False

    if _G['fb_multi']:
        xb = np.zeros((S, NPADDED, C_ATOM), BF16)
        xb[:, PAD:PAD + N] = np.asarray(atom_single, np.float32)[0].astype(BF16)
        xs_g = np.ascontiguousarray(np.lib.stride_tricks.as_strided(
            xb, shape=(NCORES, S, AH, C_ATOM),
            strides=(AC * C_ATOM * 2, NPADDED * C_ATOM * 2, C_ATOM * 2, 2)))
        xpb = np.zeros((S, NPADDED, C_ATOM), BF16)
        xpb[:, PAD:PAD + N] = np.asarray(atom_proj, np.float32)[0].astype(BF16)
        xp_g = np.ascontiguousarray(np.lib.stride_tricks.as_strided(
            xpb, shape=(NCORES, S, AH, C_ATOM),
            strides=(AC * C_ATOM * 2, NPADDED * C_ATOM * 2, C_ATOM * 2, 2)))
        pair_g = np.asarray(atom_pair_local, np.float32)[0].astype(BF16)
        mp_ = np.zeros((NPADDED,), np.float32)
        mp_[PAD:PAD + N] = np.asarray(mask, np.float32)[0]
        idx = np.arange(P)[:, None] * NQ + np.arange(NK)[None, :]
        mb = INF * (mp_[idx] - 1.0)

        import jax as _jax
        xs_d = _jax.device_put(xs_g, _G['fb_shard'])
        xp_d = _jax.device_put(xp_g, _G['fb_shard'])
        pair_d = _jax.device_put(pair_g, _G['fb_shard'])
        mb_d = _jax.device_put(mb, _G['fb_shard'])
        wd = [_jax.device_put(w, _G['fb_repl']) for w in weights]
        out = np.asarray(_G['fb_fn'](xs_d, xp_d, pair_d, mb_d, *wd))
        return out.astype(np.float32).reshape(OUT_SHAPE)

    # single-device (e.g. cpu) path: loop cores
    outs = []
    xb = np.zeros((S, NPADDED, C_ATOM), BF16)
    xb[:, PAD:PAD + N] = np.asarray(atom_single, np.float32)[0].astype(BF16)
    xpb = np.zeros((S, NPADDED, C_ATOM), BF16)
    xpb[:, PAD:PAD + N] = np.asarray(atom_proj, np.float32)[0].astype(BF16)
    pair_g = np.asarray(atom_pair_local, np.float32)[0].astype(BF16)
    mp_ = np.zeros((NPADDED,), np.float32)
    mp_[PAD:PAD + N] = np.asarray(mask, np.float32)[0]
    idx = np.arange(P)[:, None] * NQ + np.arange(NK)[None, :]
    mb = INF * (mp_[idx] - 1.0)
    for c in range(NCORES):
        lo = c * AC
        o = _G['fb_fn'](xb[:, lo:lo + AH], xpb[:, lo:lo + AH],
                        pair_g[c * WC:(c + 1) * WC], mb[c * WC:(c + 1) * WC],
                        *weights)
        outs.append(np.asarray(o))
    out = np.concatenate(outs, axis=1)
    return out.astype(np.float32).reshape(OUT_SHAPE)


def kernel(atom_single, atom_proj, atom_pair_local, mask,
           adaln_s_scale, w_gate, b_gate, w_skip,
           wq, wk, wv, wg, bg, wo, bo,
           pair_ln_scale, pair_ln_bias, w_pair, w_out, b_out):
    weights = [np.asarray(w, np.float32) for w in
               (adaln_s_scale, w_gate, b_gate, w_skip,
                wq, wk, wv, wg, bg, wo, bo,
                pair_ln_scale, pair_ln_bias, w_pair, w_out, b_out)]

    if not os.environ.get("KERNEL_NO_POOL"):
        try:
            if 'pool' not in _G:
                _tlog("spawning pool")
                _G['pool'] = _Pool()
            if _G['pool'].alive():
                return _run_pool(_G['pool'], atom_single, atom_proj,
                                 atom_pair_local, mask, weights)
        except Exception:
            traceback.print_exc()
            _G.pop('pool', None)

    return _run_fallback(atom_single, atom_proj, atom_pair_local, mask, weights)
